# revision 27
# baseline (speedup 1.0000x reference)
"""Minibatch discrimination kernel for 8 Trainium2 NeuronCores.

Math (reference):
    M = einsum('bi,iok->bok', x, T)            # [B, O, K]
    norm[i, j, o] = sum_k |M[i,o,k] - M[j,o,k]|
    out[j, o] = sum_i exp(-norm[i,j,o]) - 1.0  # [B, O]

Strategy:
  - SPMD across 8 cores; core c receives x rotated by -64*c rows and
    transposed on the host: XT = roll(x, -64c).T[:, :320]. Only 320
    b-columns are ever touched per core (windows reach j+256 <= 319).
  - Symmetry: each unordered pair {a, b} is evaluated once. Core c, local
    row j (j = 0..63), covers i in the ring window [j+1, j+256]. Row sums
    (OB) include the full window; the shared d=256 column is counted by
    both partner cores' OB (once for each of its two rows) and therefore
    excluded from the column-sum accumulators (EACD/EACP).
  - MT = (x @ T)^T per o-group: [125 = (25 o, 5 k) o-major, 320 b],
    computed with float32r matmuls (1 col/cycle vs fp32's 4) and stored
    in f16; even- and odd-shifted copies keep every window read 4B
    aligned (HW-measured: required for the DVE 2x perf mode).
  - Per j: dg = |MT16 - MT16[:, j]| for 3 groups on the DVE via a custom
    ABSOLUTE_DIFF op whose four perf-mode uop programs are cloned from
    the stock tensor_scalar tables (the fp32 per-partition scalar caps
    the RTL at 2x; measured 245ns per [125, 256] op), and 1 group on the
    ScalarE as Abs(-win + m_j) (~423ns). k-reduce on the PE: one +BO f16
    matmul per group into the 4 col-strips (tile_position=(0,32g)) into
    a shared 4-j PSUM tile. The BO strips stay resident in the PE array
    (nothing else loads weights in the j-loop -> no ldweights thrash).
  - One batched ACT Exp per 4 j's ([128, 1024], no accum_out - the
    accumulator read costs ~280ns/instr), DMA'd to DRAM as EP.
  - The row sums (OB) and shifted column sums (EAC) are computed on the
    host from EP during assembly; end-to-end rel_norm ~7e-3 < 2e-2.
"""

import numpy as np

import concourse.bacc as bacc
import concourse.mybir as mybir
from concourse.tile import TileContext
from concourse.bass_utils import run_bass_kernel_spmd

B = 512
IN_F = 512
O = 100
K = 5
NCORES = 8
JB = B // NCORES          # 64 output rows per core
NG = 4                    # o-groups
OG = O // NG              # 25 o's per group
PG = OG * K               # 125 partitions per group
W = 256                   # ring window width (d = 1..256)
XB = 320                  # b-columns touched per core (63 + 256 + 1)
EACW = 318                # eac columns 1..318 (d <= 255, j <= 63)
F32 = mybir.dt.float32
F16 = mybir.dt.float16

JEXP = 4                  # j's batched per Exp activation / output DMA
PIPE = 1                  # extra 4-j blocks in flight ahead of consumers
WB, NB, EB = 12, 3, 3     # work/psum/exp tile-pool depths


def _stock_ts_clone(op0, op1):
    """Clone the stock TENSOR_SCALAR_ARITH_OP uop programs (v3 table_ptr 16,
    mode slots +0/+1/+2/+3 = 1x/2x_1p/2x_2p/4x) with the instruction-supplied
    ALU ops replaced by concrete ones: 32 (INSTRUCTION_OP_0) -> op0,
    33 (INSTRUCTION_OP_1) -> op1. Gives the custom |in0 - s0| op the exact
    datapaths the silicon-validated stock tensor_scalar uses in every perf
    mode, including 4x (4 f16 elems/lane/cycle over both SBUF read ports)."""
    from concourse.dve_tables import find_stock_dve_bin_dir, load_table_set
    from concourse.dve_uop import (
        UopConfig, UopDpConfig, InpSel, AluInp, DelayInp, OutSel, OutPath,
        Trigger, AluOp,
    )

    ts = load_table_set(find_stock_dve_bin_dir("gen3"), "default", "v3")
    alu_sub = {32: op0, 33: op1}

    def conv(slot):
        cf = ts.control_fast[slot]
        cs = ts.control_slow[slot]
        u = UopConfig()
        u.repeat_count = cf["repeat_cnt"]
        u.next_uop = (cf["next_index0"], cf["next_index1"], cf["next_index2"])
        u.trigger = tuple(Trigger(cf[f"trigger{i}"]) for i in range(3))
        u.require_inp0 = cf["requires_src0"]
        u.require_inp1 = cf["requires_src1"]
        u.inc_parameter_index = cf["inc_parameter_index"]
        u.out_last_subdim_enable = cf["write_subdim_last"]
        u.force_two_data_zero = cf["force_two_data_zero"]
        u.force_two_data_one = cf["force_two_data_one"]
        for path, cfk, csk in (
            (OutPath.WR0_LO, "write0_en_lo", "write0_sel_lo"),
            (OutPath.WR0_HI, "write0_en_hi", "write0_sel_hi"),
            (OutPath.WR1_LO, "write1_en_lo", "write1_sel_lo"),
            (OutPath.WR1_HI, "write1_en_hi", "write1_sel_hi"),
        ):
            u.out_enable[path] = cf[cfk]
            u.out[path] = OutSel(cs[csk])
        for i in range(7):
            u.inp[i] = InpSel(cs[f"inp{i}"])
            u.inp_enable[i] = (cs["input_enable"] >> i) & 1
        u.enable_rev_ops = cs["enable_rev_ops"]
        u.match_mask = cs["match_mask"]
        u.valid_match = cs["valid_match"]
        u.replace_on_match = cs["replace_on_match"]
        u.clear_match = cs["clear_match"]
        u.write_predicate_enable = cs["write_predicate_enabled"]
        u.write_predicate_select = cs["write_predicate_select"]
        u.delay_shift8 = cs["delay_shift8"]
        u.index_increment = cs["index_increment"]
        u.index_clear = cs["index_clear"]
        blocks = []
        for be in ts.datapath[slot]:
            b = UopDpConfig()
            raw_op = be["alu_op"]
            b.op = alu_sub.get(raw_op, AluOp(raw_op) if raw_op <= 0x1C else None)
            assert b.op is not None, f"unhandled stock alu_op {raw_op}"
            b.alu_src0 = AluInp(be["mux0_sel"])
            b.alu_src1 = AluInp(be["mux1_sel"])
            b.alu_out_enable = be["out_flop_enable"]
            b.alu_out_a_enable = be["a_flop_enable"]
            b.alu_out_b_enable = be["b_flop_enable"]
            b.swap_enable = be["swap_flop_enable"]
            for i in range(6):
                b.delay[i] = DelayInp(be[f"d{i}_sel"])
                b.delay_enable[i] = be[f"d{i}_flop_enable"]
            blocks.append(b)
        u.datapath_config = blocks
        return u

    return conv(16), conv(17), conv(18), conv(19)


def _register_abs_op():
    """Register |in0 - s0| as a custom DVE op at runtime, with all four perf
    mode programs cloned from the stock tensor_scalar tables (op0 =
    ABSOLUTE_DIFF, op1 = BYPASS). perf_max=3 exposes the 4x slot: with f16
    SBUF operands the DVE runs 4 elems/lane/cycle. Idempotent."""
    import numpy as np
    from concourse import dve_ops as D
    from concourse.dve_spec import Spec, Src0, C0, Bin
    from concourse.dve_uop import AluOp, DveOpSpec
    from dataclasses import dataclass

    name = "ABS_SUB4X_MBD"
    for op in D.OPS:
        if op.name == name:
            return op
    spec = Spec(
        body=Bin(AluOp.ABSOLUTE_DIFF, Src0, C0),
        reference=lambda in0, in1, s0, s1, imm2: np.abs(
            in0.astype(np.float32) - s0),
    )
    u1, u2, u2p, u4 = _stock_ts_clone(AluOp.ABSOLUTE_DIFF, AluOp.BYPASS)

    @dataclass(frozen=True)
    class _AbsOp4x(D.DveOp):
        def compile(self, ver):
            key = (self.name, ver)
            if (r := D._COMPILE_CACHE.get(key)) is not None:
                return r
            result = DveOpSpec(
                name=self.name,
                opcode=D.get_dve_sub_opcode(self.name),
                uops=[u1],
                uops_2x=[u2],
                uops_2x_2p=[u2p],
                uops_4x=[u4],
                perf_max=3,
                rd1_en=False,
            )
            D._COMPILE_CACHE[key] = result
            return result

    row = D._CUSTOM_DVE_ROW_BASE + len(D.OPS)
    assert row < 0x20
    D._SUB_OPCODE_FOR_NAME[name] = row
    op = _AbsOp4x(name, spec, subdim=False, uops_sha={})
    D.OPS.append(op)
    D.CUSTOM_DVE_SPECS[name] = spec
    return op


ABS_OP = _register_abs_op()


def _emit_abs(nc, out, in0, s0):
    """Emit the custom |in0 - s0| op with perf_max=3. s0 may be a
    per-partition AP (caps the DVE at 2x: the fp32 scalar mem-pattern fails
    the 4x RTL trigger) or a float immediate (4x reachable for f16 SBUF
    operands - measured ~140ns vs ~245ns for the AP form at [125, 256])."""
    import concourse.bass_isa as bass_isa
    from concourse.dve_ops import get_dve_sub_opcode
    from concourse.dve_table_gen import dve_ver_for

    v = nc.vector
    b = v.bass
    op = ABS_OP
    if op.name not in b.m.ant_custom_dve_ops:
        b.m.ant_custom_dve_ops = sorted({*b.m.ant_custom_dve_ops, op.name})
    op.compile(dve_ver_for(b.trn_type))
    shape = bass_isa.CustomDveShape.TTSS
    isa_opcode = b.isa.Opcode[
        f"NEURON_ISA_TPB_OPCODE_CUSTOM_DVE_ANT_{shape.slot()}"].value
    s0_l = (mybir.ImmediateValue(dtype=mybir.dt.float32, value=float(s0))
            if isinstance(s0, (int, float)) else v.lower_ap(s0, for_isa=True))
    ins = [
        v.lower_ap(in0, for_isa=True, opt=True),
        s0_l,
        mybir.ImmediateValue(dtype=mybir.dt.float32, value=0.0),
    ]
    outs = [v.lower_ap(out, for_isa=True, opt=True)]
    return v.add_instruction(
        bass_isa.InstCustomDveAnt(
            name=b.get_next_instruction_name(),
            op_name=op.name,
            rd1_en=False,
            subdim=0,
            imm2=0.0,
            shape=shape,
            row=get_dve_sub_opcode(op.name),
            isa_opcode=isa_opcode,
            perf_max=3,
            ins=ins,
            outs=outs,
        )
    )


def _build_nc(hw_loop=0):
    nc = bacc.Bacc()

    # float32r: same 4-byte layout as fp32, but the PE streams it at 1
    # col/cycle (vs fp32's 4) for out widths >= 256 - 4x faster MT setup.
    F32R = mybir.dt.float32r
    xt = nc.declare_dram_parameter("XT", [IN_F, XB], F32R, isOutput=False)
    tt = nc.declare_dram_parameter("TT", [IN_F, O * K], F32R, isOutput=False)
    bo = nc.declare_dram_parameter("BO", [PG, 32], F16, isOutput=False)
    ep_d = nc.declare_dram_parameter("EP", [128, JB * W], F16,
                                     isOutput=True)  # [128, 64*256]

    with TileContext(nc) as tc:
        with (
            tc.tile_pool(name="const", bufs=1) as cpool,
            tc.tile_pool(name="work", bufs=WB) as wpool,
            tc.tile_pool(name="mps", bufs=1, space="PSUM") as mpspool,
            tc.tile_pool(name="nps", bufs=NB, space="PSUM") as npspool,
            tc.tile_pool(name="eps", bufs=EB) as epspool,
        ):
            bo_sb = cpool.tile([PG, 32], F16, name="bo_sb")
            nc.sync.dma_start(out=bo_sb[:], in_=bo[:])

            # Warm the PE's HAM clock gate (cold = 1.2 GHz, warm = 2.4 GHz;
            # ~3.4us of sustained activity un-throttles it) with small
            # matmuls that depend only on the first tiny DMA - they overlap
            # the XT/TT input DMAs, so the MT matmuls start at full clock.
            warm_ps = npspool.tile([128, JEXP * W], F32, name="np4", tag="norm")
            for _ in range(44):
                nc.tensor.matmul(
                    warm_ps[0:32, 0:32], bo_sb[:], bo_sb[:],
                    start=True, stop=True, skip_group_check=True)
            nc.vector.tensor_copy(ob_warm := cpool.tile(
                [32, 32], F32, name="warm_sink"), warm_ps[0:32, 0:32])

            t_sb = []
            x_sb = []
            for it in range(4):
                ts = cpool.tile([128, O * K], F32R, name=f"t_sb{it}", tag=f"t{it}")
                nc.sync.dma_start(out=ts[:], in_=tt[it * 128:(it + 1) * 128, :])
                t_sb.append(ts)
                xs = cpool.tile([128, XB], F32R, name=f"x_sb{it}", tag=f"x{it}")
                nc.sync.dma_start(out=xs[:], in_=xt[it * 128:(it + 1) * 128, :])
                x_sb.append(xs)

            # MT per group: [125 = (o_l, k) o-major, 320 b] in f16
            mt_sb = []
            for g in range(NG):
                mp = mpspool.tile([PG, XB], F32, name="mp", tag="mp")
                for it in range(4):
                    nc.tensor.matmul(
                        mp[:],
                        t_sb[it][:, g * PG:(g + 1) * PG],
                        x_sb[it][:],
                        start=(it == 0),
                        stop=(it == 3),
                    )
                # Two f16 copies of MT, element-offset by one column: the DVE
                # 2x/4x perf modes require 4-byte-aligned operand starts, and
                # window starts w0 = j+1 alternate parity. Window reads come
                # from the parity-matched copy so the start element is always
                # even. mg_e[c] = MT[c]; mg_o[c] = MT[c+1].
                mg_e = cpool.tile([PG, XB], F16, name=f"mt_e{g}", tag=f"mte{g}")
                mg_o = cpool.tile([PG, XB], F16, name=f"mt_o{g}", tag=f"mto{g}")
                if g % 2 == 0:
                    nc.vector.tensor_copy(mg_e[:], mp[:])
                    nc.scalar.copy(mg_o[:, 0:XB - 1], mp[:, 1:XB])
                else:
                    nc.scalar.copy(mg_e[:], mp[:])
                    nc.vector.tensor_copy(mg_o[:, 0:XB - 1], mp[:, 1:XB])
                # fp32 view of the f16-quantized j-columns (the scalar operand
                # path wants fp32; re-expanding the f16 values keeps the
                # subtrahend on the same quantization grid as the windows).
                mj = cpool.tile([PG, JB], F32, name=f"mtj{g}", tag=f"mtj{g}")
                if g % 2 == 0:
                    nc.scalar.copy(mj[:], mg_e[:, 0:JB])
                else:
                    nc.vector.tensor_copy(mj[:], mg_e[:, 0:JB])
                mt_sb.append((mg_e, mg_o, mj))

            def emit_producers(j, np4):
                # 4 abs + 4 k-reduce matmuls for local row j; np column block
                # (j % JEXP) of the shared 4-j PSUM tile np4.
                w0 = j + 1
                c0 = (j % JEXP) * W
                for g in range(NG):
                    mg_e, mg_o, mj = mt_sb[g]
                    win = (mg_e[:, w0:w0 + W] if w0 % 2 == 0
                           else mg_o[:, w0 - 1:w0 - 1 + W])
                    dg = wpool.tile([PG, W], F16, name="dg", tag="dg")
                    if g < 3:
                        _emit_abs(nc, dg[:], win, mj[:, j:j + 1])
                    else:
                        # |win - m_j| = Abs(-win + m_j) on the otherwise-idle
                        # ScalarE; the affine pre-stage absorbs the negation.
                        nc.scalar.activation(
                            out=dg[:], in_=win,
                            func=mybir.ActivationFunctionType.Abs,
                            bias=mj[:, j:j + 1], scale=-1.0)
                    nc.tensor.matmul(
                        np4[32 * g:32 * g + 32, c0:c0 + W], bo_sb[:], dg[:],
                        start=True, stop=True, tile_position=(0, 32 * g),
                        skip_group_check=True)

            def emit_consumer(j0, np4):
                # One batched Exp over JEXP j's worth of norms (no accum_out:
                # row/column sums happen on the host), then DMA the exps out.
                ep = epspool.tile([128, JEXP * W], F16, name="ep", tag="exp")
                nc.scalar.activation(
                    out=ep[:], in_=np4[:],
                    func=mybir.ActivationFunctionType.Exp,
                    scale=-1.0)
                nc.sync.dma_start(
                    out=ep_d[:, j0 * W:(j0 + JEXP) * W], in_=ep[:])

            import contextlib
            loop_cm = tc.For_i(0, hw_loop, 1) if hw_loop else contextlib.nullcontext()
            with loop_cm:
                pending = []
                np4 = None
                for j in range(JB):
                    if j % JEXP == 0:
                        np4 = npspool.tile(
                            [128, JEXP * W], F32, name="np4", tag="norm")
                    emit_producers(j, np4)
                    if j % JEXP == JEXP - 1:
                        pending.append((j - JEXP + 1, np4))
                    if len(pending) > PIPE:
                        jc, npc = pending.pop(0)
                        emit_consumer(jc, npc)
                for jc, npc in pending:
                    emit_consumer(jc, npc)

    nc.compile()
    return nc


_NC_CACHE = None


def _get_nc():
    global _NC_CACHE
    if _NC_CACHE is None:
        _NC_CACHE = _build_nc()
    return _NC_CACHE


def _make_consts():
    bo = np.zeros((PG, 32), dtype=np.float16)
    for p in range(PG):
        bo[p, p // K] = 1.0
    return bo


def _in_maps(x, T):
    bo = _make_consts()
    tt = np.ascontiguousarray(np.asarray(T, np.float32).reshape(IN_F, O * K))
    maps = []
    for c in range(NCORES):
        xr = np.roll(np.asarray(x, np.float32), -JB * c, axis=0)
        maps.append({
            "XT": np.ascontiguousarray(xr.T[:, :XB]),
            "TT": tt,
            "BO": bo,
        })
    return maps


def _assemble(results):
    out = np.zeros((B, O), dtype=np.float64)
    cols = np.arange(EACW) + 1          # local b-coords 1..318
    for c in range(NCORES):
        e = results[c]["EP"].astype(np.float32).reshape(128, JB, W)
        obc = e.sum(axis=2)             # [128, JB] row sums (incl d=256 col)
        eacl = np.zeros((128, XB), dtype=np.float32)
        for j in range(JB):             # column sums, d=256 col excluded
            eacl[:, j + 1:j + W] += e[:, j, 0:W - 1]
        eacc = eacl[:, 1:1 + EACW]
        rows = (cols + JB * c) % B
        for g in range(NG):
            out[JB * c:JB * (c + 1), OG * g:OG * (g + 1)] += \
                obc[32 * g:32 * g + OG, :].T.astype(np.float64)
            out[rows, OG * g:OG * (g + 1)] += \
                eacc[32 * g:32 * g + OG, :].T.astype(np.float64)
    return out.astype(np.float32)


def kernel(x: np.ndarray, T: np.ndarray) -> np.ndarray:
    x = np.ascontiguousarray(np.asarray(x, dtype=np.float32))
    T = np.ascontiguousarray(np.asarray(T, dtype=np.float32))
    assert x.shape == (B, IN_F) and T.shape == (IN_F, O, K)

    nc = _get_nc()
    res = run_bass_kernel_spmd(nc, _in_maps(x, T), list(range(NCORES)))
    return _assemble(res.results)


if __name__ == "__main__":
    rng = np.random.default_rng(0)
    x = rng.standard_normal((B, IN_F), dtype=np.float32)
    T = rng.standard_normal((IN_F, O, K), dtype=np.float32)
    out = kernel(x, T)

    # numpy reference check
    M = (x.astype(np.float64) @ T.reshape(IN_F, O * K).astype(np.float64)
         ).reshape(B, O, K)
    norm = np.abs(M[:, None, :, :] - M[None, :, :, :]).sum(-1)
    exp = np.exp(-norm)
    ref = (exp.sum(0) - 1.0).astype(np.float32)
    rel = np.linalg.norm((out - ref).ravel()) / np.linalg.norm(ref.ravel())
    print("out", out.shape, out.dtype, "rel_norm", rel)


# revision 33
# speedup vs baseline: 1.0246x; 1.0246x over previous
"""Minibatch discrimination kernel for 8 Trainium2 NeuronCores.

Math (reference):
    M = einsum('bi,iok->bok', x, T)            # [B, O, K]
    norm[i, j, o] = sum_k |M[i,o,k] - M[j,o,k]|
    out[j, o] = sum_i exp(-norm[i,j,o]) - 1.0  # [B, O]

Strategy:
  - SPMD across 8 cores; core c receives x rotated by -64*c rows and
    transposed on the host: XT = roll(x, -64c).T[:, :320]. Only 320
    b-columns are ever touched per core (windows reach j+256 <= 319).
  - Symmetry: each unordered pair {a, b} is evaluated once. Core c, local
    row j (j = 0..63), covers i in the ring window [j+1, j+256]. Row sums
    (OB) include the full window; the shared d=256 column is counted by
    both partner cores' OB (once for each of its two rows) and therefore
    excluded from the column-sum accumulators (EACD/EACP).
  - MT = (x @ T)^T per o-group: [125 = (25 o, 5 k) o-major, 320 b],
    computed with float32r matmuls (1 col/cycle vs fp32's 4) and stored
    in f16; even- and odd-shifted copies keep every window read 4B
    aligned (HW-measured: required for the DVE 2x perf mode).
  - Per j: dg = |MT16 - MT16[:, j]| for 3 groups on the DVE via a custom
    ABSOLUTE_DIFF op whose four perf-mode uop programs are cloned from
    the stock tensor_scalar tables (the fp32 per-partition scalar caps
    the RTL at 2x; measured 245ns per [125, 256] op), and 1 group on the
    ScalarE as Abs(-win + m_j) (~423ns). k-reduce on the PE: one +BO f16
    matmul per group into the 4 col-strips (tile_position=(0,32g)) into
    a shared 4-j PSUM tile. The BO strips stay resident in the PE array
    (nothing else loads weights in the j-loop -> no ldweights thrash).
  - One batched ACT Exp per 4 j's ([128, 1024], no accum_out - the
    accumulator read costs ~280ns/instr), DMA'd to DRAM as EP.
  - The row sums (OB) and shifted column sums (EAC) are computed on the
    host from EP during assembly; end-to-end rel_norm ~7e-3 < 2e-2.
"""

import numpy as np

import concourse.bacc as bacc
import concourse.mybir as mybir
from concourse.tile import TileContext
from concourse.bass_utils import run_bass_kernel_spmd

B = 512
IN_F = 512
O = 100
K = 5
NCORES = 8
JB = B // NCORES          # 64 output rows per core
NG = 4                    # o-groups
OG = O // NG              # 25 o's per group
PG = OG * K               # 125 partitions per group
W = 256                   # ring window width (d = 1..256)
XB = 320                  # b-columns touched per core (63 + 256 + 1)
EACW = 318                # eac columns 1..318 (d <= 255, j <= 63)
F32 = mybir.dt.float32
F16 = mybir.dt.float16

JEXP = 4                  # j's batched per Exp activation / output DMA
PIPE = 1                  # extra 4-j blocks in flight ahead of consumers
WB, NB, EB = 12, 3, 3     # work/psum/exp tile-pool depths


def _stock_ts_clone(op0, op1):
    """Clone the stock TENSOR_SCALAR_ARITH_OP uop programs (v3 table_ptr 16,
    mode slots +0/+1/+2/+3 = 1x/2x_1p/2x_2p/4x) with the instruction-supplied
    ALU ops replaced by concrete ones: 32 (INSTRUCTION_OP_0) -> op0,
    33 (INSTRUCTION_OP_1) -> op1. Gives the custom |in0 - s0| op the exact
    datapaths the silicon-validated stock tensor_scalar uses in every perf
    mode, including 4x (4 f16 elems/lane/cycle over both SBUF read ports)."""
    from concourse.dve_tables import find_stock_dve_bin_dir, load_table_set
    from concourse.dve_uop import (
        UopConfig, UopDpConfig, InpSel, AluInp, DelayInp, OutSel, OutPath,
        Trigger, AluOp,
    )

    ts = load_table_set(find_stock_dve_bin_dir("gen3"), "default", "v3")
    alu_sub = {32: op0, 33: op1}

    def conv(slot):
        cf = ts.control_fast[slot]
        cs = ts.control_slow[slot]
        u = UopConfig()
        u.repeat_count = cf["repeat_cnt"]
        u.next_uop = (cf["next_index0"], cf["next_index1"], cf["next_index2"])
        u.trigger = tuple(Trigger(cf[f"trigger{i}"]) for i in range(3))
        u.require_inp0 = cf["requires_src0"]
        u.require_inp1 = cf["requires_src1"]
        u.inc_parameter_index = cf["inc_parameter_index"]
        u.out_last_subdim_enable = cf["write_subdim_last"]
        u.force_two_data_zero = cf["force_two_data_zero"]
        u.force_two_data_one = cf["force_two_data_one"]
        for path, cfk, csk in (
            (OutPath.WR0_LO, "write0_en_lo", "write0_sel_lo"),
            (OutPath.WR0_HI, "write0_en_hi", "write0_sel_hi"),
            (OutPath.WR1_LO, "write1_en_lo", "write1_sel_lo"),
            (OutPath.WR1_HI, "write1_en_hi", "write1_sel_hi"),
        ):
            u.out_enable[path] = cf[cfk]
            u.out[path] = OutSel(cs[csk])
        for i in range(7):
            u.inp[i] = InpSel(cs[f"inp{i}"])
            u.inp_enable[i] = (cs["input_enable"] >> i) & 1
        u.enable_rev_ops = cs["enable_rev_ops"]
        u.match_mask = cs["match_mask"]
        u.valid_match = cs["valid_match"]
        u.replace_on_match = cs["replace_on_match"]
        u.clear_match = cs["clear_match"]
        u.write_predicate_enable = cs["write_predicate_enabled"]
        u.write_predicate_select = cs["write_predicate_select"]
        u.delay_shift8 = cs["delay_shift8"]
        u.index_increment = cs["index_increment"]
        u.index_clear = cs["index_clear"]
        blocks = []
        for be in ts.datapath[slot]:
            b = UopDpConfig()
            raw_op = be["alu_op"]
            b.op = alu_sub.get(raw_op, AluOp(raw_op) if raw_op <= 0x1C else None)
            assert b.op is not None, f"unhandled stock alu_op {raw_op}"
            b.alu_src0 = AluInp(be["mux0_sel"])
            b.alu_src1 = AluInp(be["mux1_sel"])
            b.alu_out_enable = be["out_flop_enable"]
            b.alu_out_a_enable = be["a_flop_enable"]
            b.alu_out_b_enable = be["b_flop_enable"]
            b.swap_enable = be["swap_flop_enable"]
            for i in range(6):
                b.delay[i] = DelayInp(be[f"d{i}_sel"])
                b.delay_enable[i] = be[f"d{i}_flop_enable"]
            blocks.append(b)
        u.datapath_config = blocks
        return u

    return conv(16), conv(17), conv(18), conv(19)


def _register_abs_op():
    """Register |in0 - s0| as a custom DVE op at runtime, with all four perf
    mode programs cloned from the stock tensor_scalar tables (op0 =
    ABSOLUTE_DIFF, op1 = BYPASS). perf_max=3 exposes the 4x slot: with f16
    SBUF operands the DVE runs 4 elems/lane/cycle. Idempotent."""
    import numpy as np
    from concourse import dve_ops as D
    from concourse.dve_spec import Spec, Src0, C0, Bin
    from concourse.dve_uop import AluOp, DveOpSpec
    from dataclasses import dataclass

    name = "ABS_SUB4X_MBD"
    for op in D.OPS:
        if op.name == name:
            return op
    spec = Spec(
        body=Bin(AluOp.ABSOLUTE_DIFF, Src0, C0),
        reference=lambda in0, in1, s0, s1, imm2: np.abs(
            in0.astype(np.float32) - s0),
    )
    u1, u2, u2p, u4 = _stock_ts_clone(AluOp.ABSOLUTE_DIFF, AluOp.BYPASS)

    @dataclass(frozen=True)
    class _AbsOp4x(D.DveOp):
        def compile(self, ver):
            key = (self.name, ver)
            if (r := D._COMPILE_CACHE.get(key)) is not None:
                return r
            result = DveOpSpec(
                name=self.name,
                opcode=D.get_dve_sub_opcode(self.name),
                uops=[u1],
                uops_2x=[u2],
                uops_2x_2p=[u2p],
                uops_4x=[u4],
                perf_max=3,
                rd1_en=False,
            )
            D._COMPILE_CACHE[key] = result
            return result

    row = D._CUSTOM_DVE_ROW_BASE + len(D.OPS)
    assert row < 0x20
    D._SUB_OPCODE_FOR_NAME[name] = row
    op = _AbsOp4x(name, spec, subdim=False, uops_sha={})
    D.OPS.append(op)
    D.CUSTOM_DVE_SPECS[name] = spec
    return op


ABS_OP = _register_abs_op()


def _emit_abs(nc, out, in0, s0):
    """Emit the custom |in0 - s0| op with perf_max=3. s0 may be a
    per-partition AP (caps the DVE at 2x: the fp32 scalar mem-pattern fails
    the 4x RTL trigger) or a float immediate (4x reachable for f16 SBUF
    operands - measured ~140ns vs ~245ns for the AP form at [125, 256])."""
    import concourse.bass_isa as bass_isa
    from concourse.dve_ops import get_dve_sub_opcode
    from concourse.dve_table_gen import dve_ver_for

    v = nc.vector
    b = v.bass
    op = ABS_OP
    if op.name not in b.m.ant_custom_dve_ops:
        b.m.ant_custom_dve_ops = sorted({*b.m.ant_custom_dve_ops, op.name})
    op.compile(dve_ver_for(b.trn_type))
    shape = bass_isa.CustomDveShape.TTSS
    isa_opcode = b.isa.Opcode[
        f"NEURON_ISA_TPB_OPCODE_CUSTOM_DVE_ANT_{shape.slot()}"].value
    s0_l = (mybir.ImmediateValue(dtype=mybir.dt.float32, value=float(s0))
            if isinstance(s0, (int, float)) else v.lower_ap(s0, for_isa=True))
    ins = [
        v.lower_ap(in0, for_isa=True, opt=True),
        s0_l,
        mybir.ImmediateValue(dtype=mybir.dt.float32, value=0.0),
    ]
    outs = [v.lower_ap(out, for_isa=True, opt=True)]
    return v.add_instruction(
        bass_isa.InstCustomDveAnt(
            name=b.get_next_instruction_name(),
            op_name=op.name,
            rd1_en=False,
            subdim=0,
            imm2=0.0,
            shape=shape,
            row=get_dve_sub_opcode(op.name),
            isa_opcode=isa_opcode,
            perf_max=3,
            ins=ins,
            outs=outs,
        )
    )


def _build_nc(hw_loop=0):
    nc = bacc.Bacc()

    # float32r: same 4-byte layout as fp32, but the PE streams it at 1
    # col/cycle (vs fp32's 4) for out widths >= 256 - 4x faster MT setup.
    # (f16 inputs were tried: the quantization through the 512-term
    # contraction pushed rel_norm to 2.1e-2, over the 2e-2 gate.)
    F32R = mybir.dt.float32r
    xt = nc.declare_dram_parameter("XT", [IN_F, XB], F32R, isOutput=False)
    tt = nc.declare_dram_parameter("TT", [IN_F, O * K], F32R, isOutput=False)
    bo = nc.declare_dram_parameter("BO", [PG, 32], F16, isOutput=False)
    ep_d = nc.declare_dram_parameter("EP", [128, JB * W], F16,
                                     isOutput=True)  # [128, 64*256]

    with TileContext(nc) as tc:
        with (
            tc.tile_pool(name="const", bufs=1) as cpool,
            tc.tile_pool(name="work", bufs=WB) as wpool,
            tc.tile_pool(name="mps", bufs=1, space="PSUM") as mpspool,
            tc.tile_pool(name="nps", bufs=NB, space="PSUM") as npspool,
            tc.tile_pool(name="eps", bufs=EB) as epspool,
        ):
            bo_sb = cpool.tile([PG, 32], F16, name="bo_sb")
            nc.sync.dma_start(out=bo_sb[:], in_=bo[:])

            # Warm the PE's HAM clock gate (cold = 1.2 GHz, warm = 2.4 GHz;
            # ~3.4us of sustained activity un-throttles it) with small
            # matmuls that depend only on the first tiny DMA - they overlap
            # the XT/TT input DMAs, so the MT matmuls start at full clock.
            warm_ps = npspool.tile([128, JEXP * W], F32, name="np4", tag="norm")
            for _ in range(44):
                nc.tensor.matmul(
                    warm_ps[0:32, 0:32], bo_sb[:], bo_sb[:],
                    start=True, stop=True, skip_group_check=True)
            nc.vector.tensor_copy(ob_warm := cpool.tile(
                [32, 32], F32, name="warm_sink"), warm_ps[0:32, 0:32])

            t_sb = []
            x_sb = []
            for it in range(4):
                ts = cpool.tile([128, O * K], F32R, name=f"t_sb{it}", tag=f"t{it}")
                nc.sync.dma_start(out=ts[:], in_=tt[it * 128:(it + 1) * 128, :])
                t_sb.append(ts)
                xs = cpool.tile([128, XB], F32R, name=f"x_sb{it}", tag=f"x{it}")
                nc.sync.dma_start(out=xs[:], in_=xt[it * 128:(it + 1) * 128, :])
                x_sb.append(xs)

            # MT per group: [125 = (o_l, k) o-major, 320 b] in f16
            mt_sb = []
            for g in range(NG):
                mp = mpspool.tile([PG, XB], F32, name="mp", tag="mp")
                for it in range(4):
                    nc.tensor.matmul(
                        mp[:],
                        t_sb[it][:, g * PG:(g + 1) * PG],
                        x_sb[it][:],
                        start=(it == 0),
                        stop=(it == 3),
                    )
                # Two f16 copies of MT, element-offset by one column: the DVE
                # 2x/4x perf modes require 4-byte-aligned operand starts, and
                # window starts w0 = j+1 alternate parity. Window reads come
                # from the parity-matched copy so the start element is always
                # even. mg_e[c] = MT[c]; mg_o[c] = MT[c+1].
                mg_e = cpool.tile([PG, XB], F16, name=f"mt_e{g}", tag=f"mte{g}")
                mg_o = cpool.tile([PG, XB], F16, name=f"mt_o{g}", tag=f"mto{g}")
                if g % 2 == 0:
                    nc.vector.tensor_copy(mg_e[:], mp[:])
                    nc.scalar.copy(mg_o[:, 0:XB - 1], mp[:, 1:XB])
                else:
                    nc.scalar.copy(mg_e[:], mp[:])
                    nc.vector.tensor_copy(mg_o[:, 0:XB - 1], mp[:, 1:XB])
                # fp32 view of the f16-quantized j-columns (the scalar operand
                # path wants fp32; re-expanding the f16 values keeps the
                # subtrahend on the same quantization grid as the windows).
                mj = cpool.tile([PG, JB], F32, name=f"mtj{g}", tag=f"mtj{g}")
                if g % 2 == 0:
                    nc.scalar.copy(mj[:], mg_e[:, 0:JB])
                else:
                    nc.vector.tensor_copy(mj[:], mg_e[:, 0:JB])
                mt_sb.append((mg_e, mg_o, mj))

            def emit_producers(j, np4):
                # 4 abs + 4 k-reduce matmuls for local row j; np column block
                # (j % JEXP) of the shared 4-j PSUM tile np4.
                w0 = j + 1
                c0 = (j % JEXP) * W
                for g in range(NG):
                    mg_e, mg_o, mj = mt_sb[g]
                    win = (mg_e[:, w0:w0 + W] if w0 % 2 == 0
                           else mg_o[:, w0 - 1:w0 - 1 + W])
                    dg = wpool.tile([PG, W], F16, name="dg", tag="dg")
                    if g < 3:
                        _emit_abs(nc, dg[:], win, mj[:, j:j + 1])
                    else:
                        # |win - m_j| = Abs(-win + m_j) on the otherwise-idle
                        # ScalarE; the affine pre-stage absorbs the negation.
                        nc.scalar.activation(
                            out=dg[:], in_=win,
                            func=mybir.ActivationFunctionType.Abs,
                            bias=mj[:, j:j + 1], scale=-1.0)
                    nc.tensor.matmul(
                        np4[32 * g:32 * g + 32, c0:c0 + W], bo_sb[:], dg[:],
                        start=True, stop=True, tile_position=(0, 32 * g),
                        skip_group_check=True)

            def emit_consumer(j0, np4):
                # One batched Exp over JEXP j's worth of norms (no accum_out:
                # row/column sums happen on the host), then DMA the exps out.
                ep = epspool.tile([128, JEXP * W], F16, name="ep", tag="exp")
                nc.scalar.activation(
                    out=ep[:], in_=np4[:],
                    func=mybir.ActivationFunctionType.Exp,
                    scale=-1.0)
                nc.sync.dma_start(
                    out=ep_d[:, j0 * W:(j0 + JEXP) * W], in_=ep[:])

            import contextlib
            loop_cm = tc.For_i(0, hw_loop, 1) if hw_loop else contextlib.nullcontext()
            with loop_cm:
                pending = []
                np4 = None
                for j in range(JB):
                    if j % JEXP == 0:
                        np4 = npspool.tile(
                            [128, JEXP * W], F32, name="np4", tag="norm")
                    emit_producers(j, np4)
                    if j % JEXP == JEXP - 1:
                        pending.append((j - JEXP + 1, np4))
                    if len(pending) > PIPE:
                        jc, npc = pending.pop(0)
                        emit_consumer(jc, npc)
                for jc, npc in pending:
                    emit_consumer(jc, npc)

    nc.compile()
    return nc


_NC_CACHE = None


def _get_nc():
    global _NC_CACHE
    if _NC_CACHE is None:
        _NC_CACHE = _build_nc()
    return _NC_CACHE


def _make_consts():
    bo = np.zeros((PG, 32), dtype=np.float16)
    for p in range(PG):
        bo[p, p // K] = 1.0
    return bo


def _in_maps(x, T):
    bo = _make_consts()
    tt = np.ascontiguousarray(np.asarray(T, np.float32).reshape(IN_F, O * K))
    maps = []
    for c in range(NCORES):
        xr = np.roll(np.asarray(x, np.float32), -JB * c, axis=0)
        maps.append({
            "XT": np.ascontiguousarray(xr.T[:, :XB]),
            "TT": tt,
            "BO": bo,
        })
    return maps


def _assemble(results):
    out = np.zeros((B, O), dtype=np.float64)
    cols = np.arange(EACW) + 1          # local b-coords 1..318
    for c in range(NCORES):
        e = results[c]["EP"].astype(np.float32).reshape(128, JB, W)
        obc = e.sum(axis=2)             # [128, JB] row sums (incl d=256 col)
        eacl = np.zeros((128, XB), dtype=np.float32)
        for j in range(JB):             # column sums, d=256 col excluded
            eacl[:, j + 1:j + W] += e[:, j, 0:W - 1]
        eacc = eacl[:, 1:1 + EACW]
        rows = (cols + JB * c) % B
        for g in range(NG):
            out[JB * c:JB * (c + 1), OG * g:OG * (g + 1)] += \
                obc[32 * g:32 * g + OG, :].T.astype(np.float64)
            out[rows, OG * g:OG * (g + 1)] += \
                eacc[32 * g:32 * g + OG, :].T.astype(np.float64)
    return out.astype(np.float32)


def kernel(x: np.ndarray, T: np.ndarray) -> np.ndarray:
    x = np.ascontiguousarray(np.asarray(x, dtype=np.float32))
    T = np.ascontiguousarray(np.asarray(T, dtype=np.float32))
    assert x.shape == (B, IN_F) and T.shape == (IN_F, O, K)

    nc = _get_nc()
    res = run_bass_kernel_spmd(nc, _in_maps(x, T), list(range(NCORES)))
    return _assemble(res.results)


if __name__ == "__main__":
    rng = np.random.default_rng(0)
    x = rng.standard_normal((B, IN_F), dtype=np.float32)
    T = rng.standard_normal((IN_F, O, K), dtype=np.float32)
    out = kernel(x, T)

    # numpy reference check
    M = (x.astype(np.float64) @ T.reshape(IN_F, O * K).astype(np.float64)
         ).reshape(B, O, K)
    norm = np.abs(M[:, None, :, :] - M[None, :, :, :]).sum(-1)
    exp = np.exp(-norm)
    ref = (exp.sum(0) - 1.0).astype(np.float32)
    rel = np.linalg.norm((out - ref).ravel()) / np.linalg.norm(ref.ravel())
    print("out", out.shape, out.dtype, "rel_norm", rel)
